# revision 59
# baseline (speedup 1.0000x reference)
"""Trainium2 Bass kernel for nn_MHSA_CGLU (PSA attention + Convolutional GLU).

Sharding: data-parallel over batch (B=8) across 8 NeuronCores, one batch each.
Activations live in [channels, N=H*W] layout (channels on SBUF partitions).

Schedule (decided by trace analysis; scalar-engine exp is the wall):
- 64 softmax Exp tiles (~1.15us each @1.2GHz) bound the attention phase; the
  whole kernel pipelines scores (PE) -> exp (ACT) -> o-matmul (PE) per
  (head-pair, key-tile), with v^T transposes, leftover qkv tiles, the
  pe-branch dwconv (on the DVE) and the softmax-normalize/proj of finished
  head groups all stuffed into PE/DVE stall slots inside the exp window.
- qkv packs 4 heads per M-tile at partition offsets {0,32,64,96}; a pair's
  score matmuls run concurrently in distinct PE row-groups (K=16 row tiling).
- Neither LN materializes a normalized tensor for the matmul path: qkv/fc1
  consume x directly; the psum accumulates W@x + rowsum(W) x (-mu) via a K=1
  rank row, and the rstd scale is applied on the DVE evacuation (LN affine
  pre-folded into the weights host-side).
- k-bias is dropped (softmax-invariant), q-bias rides the evacuation,
  v/BN/proj biases are host-folded into the proj bias row.
- softmax denominators come from a ones-column in vT (33rd lane per head),
  inverted with the 1-op DVE reciprocal_approx_fast, broadcast by a K=4
  matmul; cancelling +w/-w "heater" matmuls keep the PE activity monitor
  busy so the array holds its 2.4 GHz clock through dependency waits.
- PSUM: scores 3x[128,1024] (6 banks) + o-accumulator [128,1024] (2 banks).
"""

import ml_dtypes
import numpy as np

import concourse.bass as bass  # noqa: F401
import concourse.mybir as mybir
import concourse.tile as tile
from concourse import bacc
from concourse.bass_utils import run_bass_kernel_spmd

F32 = mybir.dt.float32
F32R = mybir.dt.float32r
BF16 = mybir.dt.bfloat16
AF = mybir.ActivationFunctionType
OP = mybir.AluOpType

EPS = 1e-5
NH, KD, HD = 8, 16, 32
C, N, HH, WW = 256, 1024, 32, 32
HID = 170
SCALE = KD ** -0.5


# --------------------------------------------------------------------------
# Host-side parameter folding
# --------------------------------------------------------------------------

def _bn_fold(p):
    g, b, m, v = [np.asarray(a, np.float64) for a in p]
    s = g / np.sqrt(v + EPS)
    return s, b - s * m


def fold_consts(inp):
    f64 = lambda a: np.asarray(a, np.float64)
    ln1_g, ln1_b = f64(inp["ln1_g"]), f64(inp["ln1_b"])
    ln2_g, ln2_b = f64(inp["ln2_g"]), f64(inp["ln2_b"])

    # qkv conv + BN, with LN1 affine folded in.
    s_qkv, b_qkv = _bn_fold(inp["qkv_bn"])
    Wq = s_qkv[:, None] * f64(inp["qkv_w"])          # [512, 256]
    bq = b_qkv.copy()
    bq += Wq @ ln1_b
    Wq = Wq * ln1_g[None, :]

    q_rows = np.concatenate([np.arange(64 * h, 64 * h + 16) for h in range(NH)])
    k_rows = q_rows + 16
    v_rows = np.concatenate([np.arange(64 * h + 32, 64 * h + 64) for h in range(NH)])
    Wq_q, bq_q = Wq[q_rows] * SCALE, bq[q_rows] * SCALE  # softmax scale into q
    Wq_k = Wq[k_rows]                                    # k bias dropped (softmax-inv)
    Wq_v, bq_v = Wq[v_rows], bq[v_rows]                  # v bias deferred via proj

    # qkv weight: M-tiles 0=q0 1=k0 2=q1 3=k1 4=v0 5=v1; 4 heads per q/k tile
    # at partition offsets {0,32,64,96} (16 rows each, rest zero).
    Wfull = np.zeros((6 * 128, 256))
    bqcol = np.zeros((128, 2))   # q bias per (partition, q-tile)
    for h in range(NH):
        T, j = divmod(h, 4)
        Wfull[2 * T * 128 + 32 * j: 2 * T * 128 + 32 * j + 16] = Wq_q[16 * h: 16 * h + 16]
        Wfull[(2 * T + 1) * 128 + 32 * j: (2 * T + 1) * 128 + 32 * j + 16] = Wq_k[16 * h: 16 * h + 16]
        bqcol[32 * j: 32 * j + 16, T] = bq_q[16 * h: 16 * h + 16]
    Wfull[4 * 128: 6 * 128] = Wq_v
    wqkvT = np.ascontiguousarray(Wfull.T.reshape(2, 128, 768).transpose(1, 0, 2))


    # pe branch: dwconv3x3(v) + BN (BN scale folded into taps)
    s_pe, b_pe = _bn_fold(inp["pe_bn"])
    taps_pe = s_pe[:, None, None] * f64(inp["pe_w"])[:, 0]   # [256, 3, 3]
    bfold_pe = b_pe + bq_v

    # proj conv + BN
    s_pr, b_pr = _bn_fold(inp["proj_bn"])
    Wpr = s_pr[:, None] * f64(inp["proj_w"])
    bias_proj = b_pr + Wpr @ bfold_pe
    wprojT = np.ascontiguousarray(Wpr.T.reshape(2, 128, 256).transpose(1, 0, 2))

    # fc1 with LN2 affine folded; M-tiles A1(128) A2(42) G1(128) G2(42)
    W1 = f64(inp["fc1_w"])
    b1 = f64(inp["fc1_b"]) + W1 @ ln2_b
    W1 = W1 * ln2_g[None, :]
    W1cols = np.zeros((256, 512))
    b1cols = np.zeros(512)
    W1cols[:, 0:128] = W1[0:128].T;        b1cols[0:128] = b1[0:128]
    W1cols[:, 128:170] = W1[128:170].T;    b1cols[128:170] = b1[128:170]
    W1cols[:, 256:384] = W1[170:298].T;    b1cols[256:384] = b1[170:298]
    W1cols[:, 384:426] = W1[298:340].T;    b1cols[384:426] = b1[298:340]
    wfc1T = np.ascontiguousarray(W1cols.reshape(2, 128, 512).transpose(1, 0, 2))
    w1sum = W1cols.sum(axis=0)                                # [512] rank-1 lhsT

    # GLU dwconv taps + bias
    taps_dw = f64(inp["dw_w"])[:, 0]                           # [170, 3, 3]
    b_dw = f64(inp["dw_b"])

    # fc2; K-tiles = a rows 0..127 / 128..169
    W2 = f64(inp["fc2_w"])                                     # [256, 170]
    W2T = np.zeros((2, 128, 256))
    W2T[0] = W2[:, 0:128].T
    W2T[1, 0:42] = W2[:, 128:170].T
    wfc2T = np.ascontiguousarray(W2T.transpose(1, 0, 2))       # [128, 2, 256]
    bfin = f64(inp["fc2_b"]) + ln2_b

    # pe taps as per-partition columns (DVE path); glu taps as diag (PE path)
    tpecol = np.zeros((128, 2, 9))
    ddw = np.zeros((128, 2, 9, 128))
    for t in range(2):
        for tap in range(9):
            dy, dx = divmod(tap, 3)
            tpecol[:, t, tap] = taps_pe[128 * t: 128 * t + 128, dy, dx]
    for tap in range(9):
        dy, dx = divmod(tap, 3)
        np.fill_diagonal(ddw[:, 0, tap, :], taps_dw[0:128, dy, dx])
        d1 = np.zeros(128)
        d1[0:42] = taps_dw[128:170, dy, dx]
        np.fill_diagonal(ddw[:, 1, tap, :], d1)

    # per-partition vectors [128, 14]:
    # 0,1 bq_q(t); 2,3 bv(t); 4,5 b_dw(t); 6,7 bfin(t); 8..11 b1(A1,A2,G1,G2);
    # 12,13 g2(t)
    pvec = np.zeros((128, 14))
    pvec[:, 0:2] = bqcol
    pvec[:, 2], pvec[:, 3] = bq_v[0:128], bq_v[128:256]
    pvec[0:128, 4] = b_dw[0:128]
    pvec[0:42, 5] = b_dw[128:170]
    pvec[:, 6], pvec[:, 7] = bfin[0:128], bfin[128:256]
    pvec[0:128, 8] = b1cols[0:128]
    pvec[0:42, 9] = b1cols[128:170]
    pvec[0:128, 10] = b1cols[256:384]
    pvec[0:42, 11] = b1cols[384:426]
    pvec[:, 12], pvec[:, 13] = ln2_g[0:128], ln2_g[128:256]

    rng = np.random.default_rng(7)
    hw = 0.5 * rng.standard_normal((128, 64))

    # head -> partition-group broadcast indicator for recipB (per group of 4)
    indg = np.zeros((4, 128))
    for j in range(4):
        indg[j, 32 * j: 32 * j + 32] = 1.0
    indg_p = np.zeros((2, 2, 128))   # [j(partition), pp, col]
    for pp in range(2):
        for j in range(2):
            indg_p[j, pp, 64 * pp + 32 * j: 64 * pp + 32 * j + 32] = 1.0

    f32 = lambda a: np.ascontiguousarray(a, dtype=np.float32)
    bf16 = lambda a: np.ascontiguousarray(a, dtype=ml_dtypes.bfloat16)
    return {
        "wqkvT": f32(wqkvT), "wqsumrow": f32(Wfull.sum(axis=1).reshape(1, 768)),
        "wprojT": bf16(wprojT), "bprojrow": f32(bias_proj.reshape(1, 256)),
        "wfc1T": f32(wfc1T), "w1sumrow": f32(w1sum.reshape(1, 512)),
        "wfc2T": bf16(wfc2T), "g2row": f32(np.asarray(ln2_g).reshape(1, 256)),
        "tpecol": f32(tpecol), "ddw": f32(ddw),
        "pvec": f32(pvec), "indg": f32(indg), "indg_p": f32(indg_p),
        "negc": f32(np.full((128, 1), -1.0 / C)),
        "posc": f32(np.full((128, 1), 1.0 / C)),
        "ones_rowf": f32(np.ones((1, 1024))),
        "epsrow": f32(np.full((1, 1), EPS)),
        "hw": bf16(hw), "hwn": bf16(-hw), "id128": bf16(np.eye(128)),
    }


# --------------------------------------------------------------------------
# Device program (one core, one batch)
# --------------------------------------------------------------------------

CONST_SPECS = [
    ("wqkvT", [128, 2, 768], F32R), ("wqsumrow", [1, 768], F32R),
    ("wprojT", [128, 2, 256], BF16), ("bprojrow", [1, 256], F32R),
    ("wfc1T", [128, 2, 512], F32R), ("w1sumrow", [1, 512], F32R),
    ("wfc2T", [128, 2, 256], BF16), ("g2row", [1, 256], F32R),
    ("tpecol", [128, 2, 9], F32),
    ("ddw", [128, 2, 9, 128], F32R),
    ("pvec", [128, 14], F32), ("indg", [4, 128], F32R),
    ("indg_p", [2, 2, 128], F32R),
    ("negc", [128, 1], F32R), ("posc", [128, 1], F32R),
    ("ones_rowf", [1, 1024], F32R),
    ("epsrow", [1, 1], F32),
    ("hw", [128, 64], BF16), ("hwn", [128, 64], BF16),
    ("id128", [128, 128], BF16),
]


def _ln_stats(nc, psP, work, x_tiles, xsq, consts, tagp):
    """Compute LN stats over channel dim (2 partition tiles).
    Returns (negmu_sb [1,N] f32r, Arow [1,N] f32r, Abc_ps [128,N] psum)."""
    for t in range(2):
        nc.scalar.activation(xsq[t][:], x_tiles[t][:], AF.Square)
    msb_ps = psP.tile([1, N], F32, tag="ps", name=f"{tagp}msb")
    esq_ps = psP.tile([1, N], F32, tag="ps", name=f"{tagp}esq")
    for c in range(2):
        sl = slice(c * 512, (c + 1) * 512)
        for t in range(2):
            nc.tensor.matmul(msb_ps[:, sl], consts["negc"][:], x_tiles[t][:, sl],
                             start=(t == 0), stop=(t == 1))
    for c in range(2):
        sl = slice(c * 512, (c + 1) * 512)
        for t in range(2):
            nc.tensor.matmul(esq_ps[:, sl], consts["posc"][:], xsq[t][:, sl],
                             start=(t == 0), stop=(t == 1))
    negmu = work.tile([1, N], F32R, tag="ln_negmu", name=f"{tagp}negmu")
    nc.scalar.copy(negmu[:], msb_ps[:])
    sq = work.tile([1, N], F32, tag="ln_sq", name=f"{tagp}sq")
    nc.scalar.activation(sq[:], negmu[:], AF.Square)
    nc.vector.tensor_tensor(sq[:], esq_ps[:], sq[:], OP.subtract)   # var
    nc.scalar.activation(sq[:], sq[:], AF.Ln, bias=consts["epsrow"][:])
    Arow = work.tile([1, N], F32R, tag="ln_Arow", name=f"{tagp}Arow")
    nc.scalar.activation(Arow[:], sq[:], AF.Exp, scale=-0.5)   # rstd
    return negmu, Arow


def build(num_devices=8, debug_outs=False):
    nc = bacc.Bacc("TRN2", target_bir_lowering=False, debug=False,
                   num_devices=num_devices)

    x_d = nc.dram_tensor("x", [C, N], F32R, kind="ExternalInput")
    drams = {nm: nc.dram_tensor(nm, sh, dt, kind="ExternalInput")
             for nm, sh, dt in CONST_SPECS}
    y_d = nc.dram_tensor("y", [C, N], F32, kind="ExternalOutput")
    dbg = {}
    if debug_outs:
        for nm, sh, dt in [("d_q0", [128, N], BF16),
                           ("d_k0", [128, N], BF16), ("d_vT0", [128, 264], BF16),
                           ("d_oall0", [128, N], F32), ("d_rs0", [4, N], F32),
                           ("d_o20", [128, N], BF16), ("d_pe0", [128, N], BF16),
                           ("d_xattn0", [128, N], F32), ("d_negmu2", [1, N], F32),
                           ("d_Arow2", [1, N], F32), ("d_ag0", [128, N], BF16)]:
            dbg[nm] = nc.dram_tensor(nm, sh, dt, kind="ExternalOutput")

    with tile.TileContext(nc) as tc:
        with tc.tile_pool(name="singles", bufs=1) as singles, \
             tc.tile_pool(name="work", bufs=1) as work, \
             tc.tile_pool(name="ptp", bufs=10) as ptp, \
             tc.tile_pool(name="stg", bufs=2) as stg, \
             tc.tile_pool(name="psP", bufs=3, space="PSUM") as psP, \
             tc.tile_pool(name="psO", bufs=1, space="PSUM") as psO:

            # ---- load input first, then constants in consumption order ----
            xt = [work.tile([128, N], F32R, tag=f"x{t}", name=f"x{t}") for t in range(2)]
            for t in range(2):
                for h in range(2):
                    nc.sync.dma_start(
                        xt[t][64 * h: 64 * h + 64, :],
                        x_d.ap()[t * 128 + 64 * h: t * 128 + 64 * h + 64, :])
            early = ["negc", "posc", "epsrow", "ones_rowf", "wqkvT", "wqsumrow",
                     "pvec", "indg", "hw", "hwn", "id128", "tpecol"]
            consts = {}
            by_name = {nm: (nm, sh, dt) for nm, sh, dt in CONST_SPECS}
            order = [by_name[nm] for nm in early] + \
                    [s for s in CONST_SPECS if s[0] not in early]
            for nm, sh, dt in order:
                t = singles.tile(sh, dt, tag=nm, name=nm)
                nc.sync.dma_start(t[:], drams[nm].ap())
                consts[nm] = t

            pv = consts["pvec"]

            # ---- LN1 stats; qkv consumes x directly (no z1 on the critical
            # path): psum = Wq@x + wqsum*(-mu); evac scales by rstd ----
            xsq = [work.tile([128, N], F32R, tag=f"xsq{t}", name=f"xsq{t}")
                   for t in range(2)]
            negmu1, Arow1 = _ln_stats(nc, psP, work, xt, xsq, consts, "l1")

            qk_sb = {}
            vpad = [work.tile([128, 34, 34], F32R, tag=f"pad{t}", name=f"vpad{t}")
                    for t in range(2)]
            for t in range(2):
                nc.gpsimd.memset(vpad[t][:].bitcast(mybir.dt.uint32), 0)
            names6 = ["q0", "k0", "q1", "k1", "v0", "v1"]
            # (tile indices: 2=q1 3=k1 4=v0 5=v1)

            heat_i = [0]

            def emit_heat(n):
                # standalone real-data heater matmuls into the psO slot --
                # output is never read; they only keep the PE HAM window busy
                for _ in range(n):
                    ht = psO.tile([128, 512], F32, tag="po",
                                  name=f"heat{heat_i[0]}")
                    heat_i[0] += 1
                    nc.tensor.matmul(ht[0:64, :], consts["wqkvT"][:, 0, 0:64],
                                     consts["wqkvT"][:, 1, 0:512],
                                     start=True, stop=True)

            def qkv_mm(mt):
                ps = psP.tile([128, N], F32, tag="ps", name=f"qkv{mt}")
                for c in range(2):
                    sl = slice(c * 512, (c + 1) * 512)
                    for kt in range(2):
                        nc.tensor.matmul(
                            ps[:, sl], consts["wqkvT"][:, kt, mt * 128:(mt + 1) * 128],
                            xt[kt][:, sl], start=(kt == 0), stop=False)
                    nc.tensor.matmul(
                        ps[:, sl], consts["wqsumrow"][:, mt * 128:(mt + 1) * 128],
                        negmu1[:, sl], start=False, stop=True)
                return ps

            def qkv_evac(mt, ps):
                nm = names6[mt]
                if nm[0] == "q":
                    t_sb = work.tile([128, N], BF16, tag=nm, name=nm)
                    nc.vector.tensor_tensor(t_sb[:], ps[:], Abc1[:], OP.mult)
                    nc.vector.tensor_scalar(t_sb[:], t_sb[:],
                                            pv[:, int(nm[1]):int(nm[1]) + 1],
                                            None, OP.add)
                    qk_sb[nm] = t_sb
                elif nm[0] == "k":
                    t_sb = work.tile([128, N], BF16, tag=nm, name=nm)
                    nc.vector.tensor_tensor(t_sb[:], ps[:], Abc1[:], OP.mult)
                    qk_sb[nm] = t_sb
                else:
                    vt = int(nm[1])
                    t_sb = work.tile([128, N], BF16, tag=nm, name=nm)
                    nc.vector.tensor_tensor(t_sb[:], ps[:], Abc1[:], OP.mult)
                    qk_sb[nm] = t_sb
                    nc.vector.tensor_scalar(
                        vpad[vt][:, 1:33, 1:33], t_sb[:],
                        pv[:, 2 + vt:3 + vt], None, OP.add)

            # q0/k0 kt-matmuls run while the LN1 row chain computes rstd
            # (their psum groups stay open; the -mu rank row lands later);
            # heaters fill the PE waits so the array stays warm.
            def qkv_kt(mt):
                ps = psP.tile([128, N], F32, tag="ps", name=f"qkv{mt}")
                for c in range(2):
                    sl = slice(c * 512, (c + 1) * 512)
                    for kt in range(2):
                        nc.tensor.matmul(
                            ps[:, sl], consts["wqkvT"][:, kt, mt * 128:(mt + 1) * 128],
                            xt[kt][:, sl], start=(kt == 0), stop=False)
                return ps

            def qkv_rank(mt, ps):
                for c in range(2):
                    sl = slice(c * 512, (c + 1) * 512)
                    nc.tensor.matmul(
                        ps[:, sl], consts["wqsumrow"][:, mt * 128:(mt + 1) * 128],
                        negmu1[:, sl], start=False, stop=True)

            ps_q0 = qkv_kt(0)
            ps_k0 = qkv_kt(1)
            emit_heat(4)
            qkv_rank(0, ps_q0)
            qkv_rank(1, ps_k0)
            Abc1_ps = psP.tile([128, N], F32, tag="ps", name="l1Abc")
            for c in range(2):
                sl = slice(c * 512, (c + 1) * 512)
                nc.tensor.matmul(Abc1_ps[:, sl], consts["ones_rowf"][:, 0:128],
                                 Arow1[:, sl], start=True, stop=True)
            Abc1 = work.tile([128, N], F32R, tag="Abc_sb", name="Abc1")
            nc.scalar.copy(Abc1[:], Abc1_ps[:])
            qkv_evac(0, ps_q0)
            qkv_evac(1, ps_k0)
            for mt in (4, 5):   # v tiles next (vT transposes need them early)
                qkv_evac(mt, qkv_mm(mt))
            # q1/k1 (needed only from pair 2) are spread into pair-0's loop
            qkv_rest = [2, 3]

            def emit_qkv_rest():
                if qkv_rest:
                    mt = qkv_rest.pop(0)
                    qkv_evac(mt, qkv_mm(mt))

            # v^T via PE transpose of v_sb 128x128 blocks (no z1 needed)
            vT_sb = []

            def emit_vT(nt):
                t_sb = work.tile([128, 8, 33], BF16, tag=f"vT{nt}", name=f"vT{nt}")
                for vt in range(2):
                    ps = psP.tile([128, 128], BF16, tag="ps", name=f"vT{nt}{vt}")
                    nc.tensor.transpose(
                        ps[:], qk_sb[f"v{vt}"][:, nt * 128:(nt + 1) * 128],
                        consts["id128"][:])
                    nc.vector.tensor_copy(
                        t_sb[:, 4 * vt: 4 * vt + 4, 0:32], ps[:])
                nc.vector.memset(t_sb[:, :, 32:33], 1.0)
                vT_sb.append(t_sb)

            # ---- attention; pe-dwconv runs on the DVE (PE is the pacer) ----
            pe_sb = [work.tile([128, N], BF16, tag=f"pe{t}", name=f"pe{t}")
                     for t in range(2)]
            pescr = work.tile([128, N], F32, tag="pescr", name="pescr")
            dw_jobs = [(t, tap) for t in range(2) for tap in range(9)]

            def emit_dw(njobs):
                for _ in range(njobs):
                    if not dw_jobs:
                        return
                    t, tap = dw_jobs.pop(0)
                    dy, dx = divmod(tap, 3)
                    shifted = vpad[t][:, dy: dy + 32, dx: dx + 32]
                    tapc = consts["tpecol"][:, t, tap: tap + 1]
                    if tap == 0:
                        nc.vector.tensor_scalar(pescr[:], shifted, tapc,
                                                None, OP.mult)
                    elif tap < 8:
                        nc.vector.scalar_tensor_tensor(
                            pescr[:], shifted, tapc, pescr[:],
                            OP.mult, OP.add)
                    else:
                        nc.vector.scalar_tensor_tensor(
                            pe_sb[t][:], shifted, tapc, pescr[:],
                            OP.mult, OP.add)

            o_all = [work.tile([128, N], F32R, tag=f"oall{g}", name=f"oall{g}")
                     for g in range(2)]
            rs_g = [work.tile([4, N], F32, tag=f"rs{g}", name=f"rs{g}")
                    for g in range(2)]
            o2 = [work.tile([128, N], BF16, tag=f"o2{g}", name=f"o2{g}")
                  for g in range(2)]
            xacc = xt  # proj residual accumulated in place

            pts = {}

            def emit_scores(p, mt, oPS):
                T = p // 2
                jA = (2 * p) % 4
                jg = (jA + 2) % 4   # heater row-group, disjoint from the pair
                q_t, k_t = qk_sb[f"q{T}"], qk_sb[f"k{T}"]
                Ps = [psP.tile([128, N], F32, tag="ps", name=f"sc{p}_{mt}_{hh}")
                      for hh in range(2)]
                for c in range(2):
                    sl = slice(c * 512, (c + 1) * 512)
                    for hh, j in ((0, jA), (1, jA + 1)):
                        nc.tensor.matmul(
                            Ps[hh][:, sl],
                            k_t[32 * j: 32 * j + 16, mt * 128:(mt + 1) * 128],
                            q_t[32 * j: 32 * j + 16, sl],
                            start=True, stop=True, tile_position=(32 * j, 0))
                for hh in range(2):
                    pt = ptp.tile([128, N], BF16, tag="pt", name=f"pt{p}_{mt}_{hh}")
                    nc.scalar.activation(pt[:], Ps[hh][:], AF.Exp)
                    pts[(p, mt, hh)] = pt

            def emit_o(p, mt, oPS):
                hA, hB = 2 * p, 2 * p + 1
                ptA, ptB = pts.pop((p, mt, 0)), pts.pop((p, mt, 1))
                for c in range(2):
                    sl = slice(c * 512, (c + 1) * 512)
                    nc.tensor.matmul(
                        oPS[0:33, sl], vT_sb[mt][:, hA, :], ptA[:, sl],
                        start=(mt == 0), stop=(mt == 7), tile_position=(0, 0))
                    nc.tensor.matmul(
                        oPS[64:97, sl], vT_sb[mt][:, hB, :], ptB[:, sl],
                        start=(mt == 0), stop=(mt == 7), tile_position=(0, 64))

            def emit_recip_o2(g, c):
                # softmax denominators -> recip -> broadcast -> o2 = o * 1/den
                if c == 0:
                    from concourse.dve_ops import (
                        RECIP_APPROX_FAST_CONSTS, RECIPROCAL_APPROX_FAST)
                    cc = RECIP_APPROX_FAST_CONSTS
                    with nc.allow_low_precision(reason="softmax denominators"):
                        nc.vector._custom_dve(
                            RECIPROCAL_APPROX_FAST, out=rs_g[g][:].bitcast(F32R),
                            in0=rs_g[g][:], s0=cc["s0"], s1=cc["s1"],
                            imm2=cc["imm2"])
                sl = slice(c * 512, (c + 1) * 512)
                rB = psP.tile([128, 512], F32, tag="ps", name=f"recipB{g}{c}")
                nc.tensor.matmul(rB[:], consts["indg"][:],
                                 rs_g[g][:, sl].bitcast(F32R),
                                 start=True, stop=True)
                nc.vector.tensor_tensor(o2[g][:, sl], o_all[g][:, sl],
                                        rB[:], OP.mult)

            def emit_proj_g0(mt, c):
                # pe-branch (both kt) + bias + g0's o2 in one accumulation
                sl = slice(c * 512, (c + 1) * 512)
                pp = psP.tile([128, 512], F32, tag="ps", name=f"pj0{mt}{c}")
                mc = slice(mt * 128, (mt + 1) * 128)
                nc.tensor.matmul(pp[:], consts["wprojT"][:, 0, mc],
                                 pe_sb[0][:, sl], start=True, stop=False)
                nc.tensor.matmul(pp[:], consts["wprojT"][:, 1, mc],
                                 pe_sb[1][:, sl], start=False, stop=False)
                nc.tensor.matmul(pp[:], consts["bprojrow"][:, mc],
                                 consts["ones_rowf"][:, 0:512],
                                 start=False, stop=False)
                nc.tensor.matmul(pp[:], consts["wprojT"][:, 0, mc],
                                 o2[0][:, sl], start=False, stop=True)
                nc.vector.tensor_tensor(xacc[mt][:, sl], xacc[mt][:, sl],
                                        pp[:], OP.add)

            def emit_proj_g1(mt, c):
                sl = slice(c * 512, (c + 1) * 512)
                pp = psP.tile([128, 512], F32, tag="ps", name=f"pj1{mt}{c}")
                nc.tensor.matmul(pp[:], consts["wprojT"][:, 1,
                                                         mt * 128:(mt + 1) * 128],
                                 o2[1][:, sl], start=True, stop=True)
                nc.vector.tensor_tensor(xacc[mt][:, sl], xacc[mt][:, sl],
                                        pp[:], OP.add)

            def emit_group_mid0_part(i):
                if i == 0:
                    emit_recip_o2(0, 0)
                elif i == 1:
                    emit_recip_o2(0, 1)
                elif i < 6:
                    emit_proj_g0((i - 2) // 2, (i - 2) % 2)

            def emit_group_mid(g):
                assert g == 1
                for c in range(2):
                    emit_recip_o2(1, c)
                for mt in range(2):
                    for c in range(2):
                        sl = slice(c * 512, (c + 1) * 512)
                        pp = psP.tile([128, 512], F32, tag="ps",
                                      name=f"pj1{mt}{c}")
                        nc.tensor.matmul(
                            pp[:], consts["wprojT"][:, 1,
                                                    mt * 128:(mt + 1) * 128],
                            o2[1][:, sl], start=True, stop=True)
                        nc.vector.tensor_tensor(xacc[mt][:, sl],
                                                xacc[mt][:, sl],
                                                pp[:], OP.add)

            for p in range(4):
                g = p // 2
                oPS = psO.tile([128, N], F32, tag="po", name=f"oacc{p}")
                for mt in range(8):
                    emit_scores(p, mt, oPS)
                    if p == 0:
                        emit_vT(mt)
                    if mt > 0:
                        # cancelling heater pair emitted while the PE would
                        # otherwise wait on the exp gating o(mt-1); pair 3 is
                        # PE-oversubscribed by g0's mid work, so skip it there
                        for wname in (("hw", "hwn") if p < 3 else ()):
                            nc.tensor.matmul(
                                oPS[0:33, 0:512], consts[wname][:, 0:33],
                                qk_sb["q0"][:, 0:512], start=False, stop=False)
                        emit_o(p, mt - 1, oPS)
                    if p == 1:
                        emit_dw(2)
                        emit_qkv_rest()
                    elif p == 2:
                        emit_dw(1)
                    # g0's mid work spread over p3's slots (deps long ready)
                    if p == 3 and 1 <= mt <= 4:
                        emit_group_mid0_part(mt - 1)
                if p == 1:
                    emit_dw(2)
                if p == 3:
                    emit_group_mid0_part(4)
                    emit_group_mid0_part(5)
                emit_o(p, 7, oPS)
                stage = stg.tile([97, N], F32R, tag="stage", name=f"stage{p}")
                nc.vector.tensor_copy(stage[:], oPS[0:97, :])
                pp = p % 2
                nc.sync.dma_start(o_all[g][64 * pp: 64 * pp + 32, :], stage[0:32, :])
                nc.sync.dma_start(o_all[g][64 * pp + 32: 64 * pp + 64, :],
                                  stage[64:96, :])
                nc.sync.dma_start(rs_g[g][2 * pp: 2 * pp + 1, :],
                                  stage[32:33, :].bitcast(F32))
                nc.sync.dma_start(rs_g[g][2 * pp + 1: 2 * pp + 2, :],
                                  stage[96:97, :].bitcast(F32))

            emit_group_mid(1)
            x_attn = xacc

            if debug_outs:
                nc.sync.dma_start(dbg["d_q0"].ap(), qk_sb["q0"][:])
                nc.sync.dma_start(dbg["d_k0"].ap(), qk_sb["k0"][:])
                nc.sync.dma_start(dbg["d_vT0"].ap(), vT_sb[0][:])
                nc.sync.dma_start(dbg["d_oall0"].ap(), o_all[0][:].bitcast(F32))
                nc.sync.dma_start(dbg["d_rs0"].ap(), rs_g[0][:])
                nc.sync.dma_start(dbg["d_o20"].ap(), o2[0][:])
                nc.sync.dma_start(dbg["d_pe0"].ap(), pe_sb[0][:])
                nc.sync.dma_start(dbg["d_xattn0"].ap(), x_attn[0][:].bitcast(F32))

            # ---- LN2 stats (no z2 materialization) ----
            negmu2, Arow2 = _ln_stats(nc, psP, work, x_attn, xsq, consts, "l2")
            B2row = work.tile([1, N], F32R, tag="ln_B", name="B2row")
            nc.vector.tensor_tensor(B2row[:], negmu2[:], Arow2[:], OP.mult)

            if debug_outs:
                nc.sync.dma_start(dbg["d_negmu2"].ap(), negmu2[:].bitcast(F32))
                nc.sync.dma_start(dbg["d_Arow2"].ap(), Arow2[:].bitcast(F32))

            # ---- fc1: psum = W1@x + w1sum*(-mu); evac scales by rstd ----
            apad = [work.tile([128, 34, 34], F32R, tag=f"pad{t}", name=f"apad{t}")
                    for t in range(2)]
            for t in range(2):
                nc.gpsimd.memset(apad[t][:].bitcast(mybir.dt.uint32), 0)
            Gtmp = [work.tile([128, N], BF16, tag=f"Gt{t}", name=f"Gt{t}")
                    for t in range(2)]
            nparts = [128, 42, 128, 42]

            def fc1_mm(mt, np_, with_rank):
                ps = psP.tile([128, N], F32, tag="ps", name=f"fc1_{mt}")
                for c in range(2):
                    sl = slice(c * 512, (c + 1) * 512)
                    for kt in range(2):
                        nc.tensor.matmul(
                            ps[0:np_, sl],
                            consts["wfc1T"][:, kt, mt * 128: mt * 128 + np_],
                            x_attn[kt][:, sl], start=(kt == 0),
                            stop=(kt == 1 and not with_rank))
                    if with_rank:
                        nc.tensor.matmul(
                            ps[0:np_, sl],
                            consts["w1sumrow"][:, mt * 128: mt * 128 + np_],
                            negmu2[:, sl], start=False, stop=True)
                return ps

            def fc1_rank(mt, np_, ps):
                for c in range(2):
                    sl = slice(c * 512, (c + 1) * 512)
                    nc.tensor.matmul(
                        ps[0:np_, sl],
                        consts["w1sumrow"][:, mt * 128: mt * 128 + np_],
                        negmu2[:, sl], start=False, stop=True)

            def fc1_evac(mt, np_, ps):
                if mt < 2:
                    nc.vector.tensor_tensor(apad[mt][0:np_, 1:33, 1:33],
                                            ps[0:np_], Abc2[0:np_], OP.mult)
                    nc.vector.tensor_scalar(apad[mt][0:np_, 1:33, 1:33],
                                            apad[mt][0:np_, 1:33, 1:33],
                                            pv[0:np_, 8 + mt:9 + mt], None, OP.add)
                else:
                    nc.vector.tensor_tensor(Gtmp[mt - 2][0:np_], ps[0:np_],
                                            Abc2[0:np_], OP.mult)

            # A1/A2 kt-matmuls overlap the LN2 row chain (open psum groups),
            # then broadcast rstd, finish with the rank rows, evacuate.
            ps_a1 = fc1_mm(0, 128, with_rank=False)
            ps_a2 = fc1_mm(1, 42, with_rank=False)
            emit_heat(8)
            Abc2_ps = psP.tile([128, N], F32, tag="ps", name="l2Abc")
            for c in range(2):
                sl = slice(c * 512, (c + 1) * 512)
                nc.tensor.matmul(Abc2_ps[:, sl], consts["ones_rowf"][:, 0:128],
                                 Arow2[:, sl], start=True, stop=True)
            Abc2 = work.tile([128, N], F32R, tag="Abc_sb", name="Abc2")
            nc.scalar.copy(Abc2[:], Abc2_ps[:])
            fc1_rank(1, 42, ps_a2)
            fc1_rank(0, 128, ps_a1)
            fc1_evac(1, 42, ps_a2)
            fc1_evac(0, 128, ps_a1)
            for mt in (2, 3):
                np_ = nparts[mt]
                ps = fc1_mm(mt, np_, with_rank=True)
                fc1_evac(mt, np_, ps)

            # W = Abc2 * x_attn feeds y1 = (W*g2 + bfin) + x_attn (DVE, runs
            # while the GLU dwconv occupies the PE; needed only at fc2 tail)
            W = [work.tile([128, N], F32R, tag=f"xsq{t}", name=f"W{t}") for t in range(2)]
            y1 = [work.tile([128, N], F32, tag=f"oall{t}", name=f"y1{t}")
                  for t in range(2)]

            # ---- GLU dwconv on the PE (runs warm behind heaters) ----
            dwps = []
            for t in range(2):
                np_ = nparts[t]
                ps = psP.tile([128, N], F32, tag="ps", name=f"dwglu{t}")
                for c in range(2):
                    for tap in range(9):
                        dy, dx = divmod(tap, 3)
                        rhs = apad[t][0:np_, dy + 16 * c: dy + 16 * c + 16, dx: dx + 32]
                        nc.tensor.matmul(
                            ps[0:np_, c * 512:(c + 1) * 512],
                            consts["ddw"][0:np_, t, tap, 0:np_], rhs,
                            start=(tap == 0), stop=(tap == 8))
                dwps.append(ps)
            ag = []
            for t in range(2):
                np_ = nparts[t]
                a_act = work.tile([128, N], BF16, tag=f"aact{t}", name=f"aact{t}")
                nc.scalar.activation(a_act[0:np_], dwps[t][0:np_], AF.Gelu,
                                     bias=pv[0:np_, 4 + t:5 + t])
                # W/y1 DVE work overlaps the PE dwconv / ACT gelu
                nc.vector.tensor_tensor(W[t][:], x_attn[t][:], Abc2[:], OP.mult)
                nc.vector.affine_then_add(y1[t][:], W[t][:].bitcast(F32),
                                          x_attn[t][:].bitcast(F32),
                                          pv[:, 12 + t:13 + t], pv[:, 6 + t:7 + t])
                agt = work.tile([128, N], BF16, tag=f"ag{t}", name=f"ag{t}")
                nc.vector.scalar_tensor_tensor(
                    agt[0:np_], Gtmp[t][0:np_], pv[0:np_, 10 + t:11 + t],
                    a_act[0:np_], OP.add, OP.mult)
                ag.append(agt)

            if debug_outs:
                nc.sync.dma_start(dbg["d_ag0"].ap(), ag[0][:])

            # ---- fc2 (+ rank-1 g2 x B2) + final residual ----
            # kt0 contribution issues as soon as ag[0] exists; kt1 + the
            # rank row close the accumulation once ag[1] lands.
            fc2_ps = [psP.tile([128, N], F32, tag="ps", name=f"fc2_{mt}")
                      for mt in range(2)]
            for mt in range(2):
                for c in range(2):
                    sl = slice(c * 512, (c + 1) * 512)
                    nc.tensor.matmul(
                        fc2_ps[mt][:, sl],
                        consts["wfc2T"][0:128, 0, mt * 128:(mt + 1) * 128],
                        ag[0][0:128, sl], start=True, stop=False)
            for mt in range(2):
                for c in range(2):
                    sl = slice(c * 512, (c + 1) * 512)
                    nc.tensor.matmul(
                        fc2_ps[mt][:, sl],
                        consts["wfc2T"][0:42, 1, mt * 128:(mt + 1) * 128],
                        ag[1][0:42, sl], start=False, stop=False)
                    nc.tensor.matmul(
                        fc2_ps[mt][:, sl],
                        consts["g2row"][:, mt * 128:(mt + 1) * 128],
                        B2row[:, sl], start=False, stop=True)
                yt = work.tile([128, N], F32, tag=f"xsq{mt}", name=f"y{mt}")
                nc.vector.tensor_tensor(yt[:], y1[mt][:], fc2_ps[mt][:], OP.add)
                nc.sync.dma_start(y_d.ap()[mt * 128:(mt + 1) * 128, :], yt[:])

    nc.compile()
    return nc


_NC = None
_NC_DBG = None


def kernel(**inputs):
    global _NC
    consts = fold_consts(inputs)
    if _NC is None:
        _NC = build()
    x = np.asarray(inputs["x"], np.float32)
    B = x.shape[0]
    in_maps = []
    for b in range(B):
        m = dict(consts)
        m["x"] = np.ascontiguousarray(x[b].reshape(C, N))
        in_maps.append(m)
    res = run_bass_kernel_spmd(_NC, in_maps, core_ids=list(range(B)))
    out = np.stack([res.results[b]["y"].reshape(C, HH, WW) for b in range(B)])
    return out


# revision 61
# speedup vs baseline: 1.1107x; 1.1107x over previous
"""Trainium2 Bass kernel for nn_MHSA_CGLU (PSA attention + Convolutional GLU).

Sharding: data-parallel over batch (B=8) across 8 NeuronCores, one batch each.
Activations live in [channels, N=H*W] layout (channels on SBUF partitions).

Schedule (decided by trace analysis; scalar-engine exp is the wall):
- 64 softmax Exp tiles (~1.15us each @1.2GHz) bound the attention phase; the
  whole kernel pipelines scores (PE) -> exp (ACT) -> o-matmul (PE) per
  (head-pair, key-tile), with v^T transposes, leftover qkv tiles, the
  pe-branch dwconv (on the DVE) and the softmax-normalize/proj of finished
  head groups all stuffed into PE/DVE stall slots inside the exp window.
- qkv packs 4 heads per M-tile at partition offsets {0,32,64,96}; a pair's
  score matmuls run concurrently in distinct PE row-groups (K=16 row tiling).
- Neither LN materializes a normalized tensor for the matmul path: qkv/fc1
  consume x directly; the psum accumulates W@x + rowsum(W) x (-mu) via a K=1
  rank row, and the rstd scale is applied on the DVE evacuation (LN affine
  pre-folded into the weights host-side).
- k-bias is dropped (softmax-invariant), q-bias rides the evacuation,
  v/BN/proj biases are host-folded into the proj bias row.
- softmax denominators come from a ones-column in vT (33rd lane per head),
  inverted with the 1-op DVE reciprocal_approx_fast, broadcast by a K=4
  matmul; cancelling +w/-w "heater" matmuls keep the PE activity monitor
  busy so the array holds its 2.4 GHz clock through dependency waits.
- PSUM: scores 3x[128,1024] (6 banks) + o-accumulator [128,1024] (2 banks).
"""

import ml_dtypes
import numpy as np

import concourse.bass as bass  # noqa: F401
import concourse.mybir as mybir
import concourse.tile as tile
from concourse import bacc
from concourse.bass_utils import run_bass_kernel_spmd

F32 = mybir.dt.float32
F32R = mybir.dt.float32r
BF16 = mybir.dt.bfloat16
AF = mybir.ActivationFunctionType
OP = mybir.AluOpType

EPS = 1e-5
NH, KD, HD = 8, 16, 32
C, N, HH, WW = 256, 1024, 32, 32
HID = 170
SCALE = KD ** -0.5


# --------------------------------------------------------------------------
# Host-side parameter folding
# --------------------------------------------------------------------------

def _bn_fold(p):
    g, b, m, v = [np.asarray(a, np.float64) for a in p]
    s = g / np.sqrt(v + EPS)
    return s, b - s * m


def fold_consts(inp):
    f64 = lambda a: np.asarray(a, np.float64)
    ln1_g, ln1_b = f64(inp["ln1_g"]), f64(inp["ln1_b"])
    ln2_g, ln2_b = f64(inp["ln2_g"]), f64(inp["ln2_b"])

    # qkv conv + BN, with LN1 affine folded in.
    s_qkv, b_qkv = _bn_fold(inp["qkv_bn"])
    Wq = s_qkv[:, None] * f64(inp["qkv_w"])          # [512, 256]
    bq = b_qkv.copy()
    bq += Wq @ ln1_b
    Wq = Wq * ln1_g[None, :]

    q_rows = np.concatenate([np.arange(64 * h, 64 * h + 16) for h in range(NH)])
    k_rows = q_rows + 16
    v_rows = np.concatenate([np.arange(64 * h + 32, 64 * h + 64) for h in range(NH)])
    Wq_q, bq_q = Wq[q_rows] * SCALE, bq[q_rows] * SCALE  # softmax scale into q
    Wq_k = Wq[k_rows]                                    # k bias dropped (softmax-inv)
    Wq_v, bq_v = Wq[v_rows], bq[v_rows]                  # v bias deferred via proj

    # qkv weight: M-tiles 0=q0 1=k0 2=q1 3=k1 4=v0 5=v1; 4 heads per q/k tile
    # at partition offsets {0,32,64,96} (16 rows each, rest zero).
    Wfull = np.zeros((6 * 128, 256))
    bqcol = np.zeros((128, 2))   # q bias per (partition, q-tile)
    for h in range(NH):
        T, j = divmod(h, 4)
        Wfull[2 * T * 128 + 32 * j: 2 * T * 128 + 32 * j + 16] = Wq_q[16 * h: 16 * h + 16]
        Wfull[(2 * T + 1) * 128 + 32 * j: (2 * T + 1) * 128 + 32 * j + 16] = Wq_k[16 * h: 16 * h + 16]
        bqcol[32 * j: 32 * j + 16, T] = bq_q[16 * h: 16 * h + 16]
    Wfull[4 * 128: 6 * 128] = Wq_v
    wqkvT = np.ascontiguousarray(Wfull.T.reshape(2, 128, 768).transpose(1, 0, 2))


    # pe branch: dwconv3x3(v) + BN (BN scale folded into taps)
    s_pe, b_pe = _bn_fold(inp["pe_bn"])
    taps_pe = s_pe[:, None, None] * f64(inp["pe_w"])[:, 0]   # [256, 3, 3]
    bfold_pe = b_pe + bq_v

    # proj conv + BN
    s_pr, b_pr = _bn_fold(inp["proj_bn"])
    Wpr = s_pr[:, None] * f64(inp["proj_w"])
    bias_proj = b_pr + Wpr @ bfold_pe
    wprojT = np.ascontiguousarray(Wpr.T.reshape(2, 128, 256).transpose(1, 0, 2))

    # fc1 with LN2 affine folded; M-tiles A1(128) A2(42) G1(128) G2(42)
    W1 = f64(inp["fc1_w"])
    b1 = f64(inp["fc1_b"]) + W1 @ ln2_b
    W1 = W1 * ln2_g[None, :]
    W1cols = np.zeros((256, 512))
    b1cols = np.zeros(512)
    W1cols[:, 0:128] = W1[0:128].T;        b1cols[0:128] = b1[0:128]
    W1cols[:, 128:170] = W1[128:170].T;    b1cols[128:170] = b1[128:170]
    W1cols[:, 256:384] = W1[170:298].T;    b1cols[256:384] = b1[170:298]
    W1cols[:, 384:426] = W1[298:340].T;    b1cols[384:426] = b1[298:340]
    wfc1T = np.ascontiguousarray(W1cols.reshape(2, 128, 512).transpose(1, 0, 2))
    w1sum = W1cols.sum(axis=0)                                # [512] rank-1 lhsT

    # GLU dwconv taps + bias
    taps_dw = f64(inp["dw_w"])[:, 0]                           # [170, 3, 3]
    b_dw = f64(inp["dw_b"])

    # fc2; K-tiles = a rows 0..127 / 128..169
    W2 = f64(inp["fc2_w"])                                     # [256, 170]
    W2T = np.zeros((2, 128, 256))
    W2T[0] = W2[:, 0:128].T
    W2T[1, 0:42] = W2[:, 128:170].T
    wfc2T = np.ascontiguousarray(W2T.transpose(1, 0, 2))       # [128, 2, 256]
    bfin = f64(inp["fc2_b"]) + ln2_b

    # pe taps as per-partition columns (DVE path); glu taps as diag (PE path)
    tpecol = np.zeros((128, 2, 9))
    ddw = np.zeros((128, 2, 9, 128))
    for t in range(2):
        for tap in range(9):
            dy, dx = divmod(tap, 3)
            tpecol[:, t, tap] = taps_pe[128 * t: 128 * t + 128, dy, dx]
    for tap in range(9):
        dy, dx = divmod(tap, 3)
        np.fill_diagonal(ddw[:, 0, tap, :], taps_dw[0:128, dy, dx])
        d1 = np.zeros(128)
        d1[0:42] = taps_dw[128:170, dy, dx]
        np.fill_diagonal(ddw[:, 1, tap, :], d1)

    # per-partition vectors [128, 14]:
    # 0,1 bq_q(t); 2,3 bv(t); 4,5 b_dw(t); 6,7 bfin(t); 8..11 b1(A1,A2,G1,G2);
    # 12,13 g2(t)
    pvec = np.zeros((128, 14))
    pvec[:, 0:2] = bqcol
    pvec[:, 2], pvec[:, 3] = bq_v[0:128], bq_v[128:256]
    pvec[0:128, 4] = b_dw[0:128]
    pvec[0:42, 5] = b_dw[128:170]
    pvec[:, 6], pvec[:, 7] = bfin[0:128], bfin[128:256]
    pvec[0:128, 8] = b1cols[0:128]
    pvec[0:42, 9] = b1cols[128:170]
    pvec[0:128, 10] = b1cols[256:384]
    pvec[0:42, 11] = b1cols[384:426]
    pvec[:, 12], pvec[:, 13] = ln2_g[0:128], ln2_g[128:256]

    rng = np.random.default_rng(7)
    hw = 0.5 * rng.standard_normal((128, 64))

    # head -> partition-group broadcast indicator for recipB (per group of 4)
    indg = np.zeros((4, 128))
    for j in range(4):
        indg[j, 32 * j: 32 * j + 32] = 1.0
    indg_p = np.zeros((2, 2, 128))   # [j(partition), pp, col]
    for pp in range(2):
        for j in range(2):
            indg_p[j, pp, 64 * pp + 32 * j: 64 * pp + 32 * j + 32] = 1.0

    f32 = lambda a: np.ascontiguousarray(a, dtype=np.float32)
    bf16 = lambda a: np.ascontiguousarray(a, dtype=ml_dtypes.bfloat16)
    return {
        "wqkvT": f32(wqkvT), "wqsumrow": f32(Wfull.sum(axis=1).reshape(1, 768)),
        "wprojT": bf16(wprojT), "bprojrow": f32(bias_proj.reshape(1, 256)),
        "wfc1T": f32(wfc1T), "w1sumrow": f32(w1sum.reshape(1, 512)),
        "wfc2T": bf16(wfc2T), "g2row": f32(np.asarray(ln2_g).reshape(1, 256)),
        "tpecol": f32(tpecol), "ddw": f32(ddw),
        "pvec": f32(pvec), "indg": f32(indg), "indg_p": f32(indg_p),
        "negc": f32(np.full((128, 1), -1.0 / C)),
        "posc": f32(np.full((128, 1), 1.0 / C)),
        "ones_rowf": f32(np.ones((1, 1024))),
        "epsrow": f32(np.full((1, 1), EPS)),
        "hw": bf16(hw), "hwn": bf16(-hw), "id128": bf16(np.eye(128)),
    }


# --------------------------------------------------------------------------
# Device program (one core, one batch)
# --------------------------------------------------------------------------

CONST_SPECS = [
    ("wqkvT", [128, 2, 768], F32R), ("wqsumrow", [1, 768], F32R),
    ("wprojT", [128, 2, 256], BF16), ("bprojrow", [1, 256], F32R),
    ("wfc1T", [128, 2, 512], F32R), ("w1sumrow", [1, 512], F32R),
    ("wfc2T", [128, 2, 256], BF16), ("g2row", [1, 256], F32R),
    ("tpecol", [128, 2, 9], F32),
    ("ddw", [128, 2, 9, 128], F32R),
    ("pvec", [128, 14], F32), ("indg", [4, 128], F32R),
    ("indg_p", [2, 2, 128], F32R),
    ("negc", [128, 1], F32R), ("posc", [128, 1], F32R),
    ("ones_rowf", [1, 1024], F32R),
    ("epsrow", [1, 1], F32),
    ("hw", [128, 64], BF16), ("hwn", [128, 64], BF16),
    ("id128", [128, 128], BF16),
]


def _ln_stats(nc, psP, work, x_tiles, xsq, consts, tagp):
    """Compute LN stats over channel dim (2 partition tiles).
    Returns (negmu_sb [1,N] f32r, Arow [1,N] f32r, Abc_ps [128,N] psum)."""
    for t in range(2):
        nc.scalar.activation(xsq[t][:], x_tiles[t][:], AF.Square)
    msb_ps = psP.tile([1, N], F32, tag="ps", name=f"{tagp}msb")
    esq_ps = psP.tile([1, N], F32, tag="ps", name=f"{tagp}esq")
    for c in range(2):
        sl = slice(c * 512, (c + 1) * 512)
        for t in range(2):
            nc.tensor.matmul(msb_ps[:, sl], consts["negc"][:], x_tiles[t][:, sl],
                             start=(t == 0), stop=(t == 1))
    for c in range(2):
        sl = slice(c * 512, (c + 1) * 512)
        for t in range(2):
            nc.tensor.matmul(esq_ps[:, sl], consts["posc"][:], xsq[t][:, sl],
                             start=(t == 0), stop=(t == 1))
    negmu = work.tile([1, N], F32R, tag="ln_negmu", name=f"{tagp}negmu")
    nc.scalar.copy(negmu[:], msb_ps[:])
    sq = work.tile([1, N], F32, tag="ln_sq", name=f"{tagp}sq")
    nc.scalar.activation(sq[:], negmu[:], AF.Square)
    nc.vector.tensor_tensor(sq[:], esq_ps[:], sq[:], OP.subtract)   # var
    nc.scalar.activation(sq[:], sq[:], AF.Ln, bias=consts["epsrow"][:])
    Arow = work.tile([1, N], F32R, tag="ln_Arow", name=f"{tagp}Arow")
    nc.scalar.activation(Arow[:], sq[:], AF.Exp, scale=-0.5)   # rstd
    return negmu, Arow


def build(num_devices=8, debug_outs=False):
    nc = bacc.Bacc("TRN2", target_bir_lowering=False, debug=False,
                   num_devices=num_devices)

    x_d = nc.dram_tensor("x", [C, N], F32R, kind="ExternalInput")
    drams = {nm: nc.dram_tensor(nm, sh, dt, kind="ExternalInput")
             for nm, sh, dt in CONST_SPECS}
    y_d = nc.dram_tensor("y", [C, N], F32, kind="ExternalOutput")
    dbg = {}
    if debug_outs:
        for nm, sh, dt in [("d_q0", [128, N], BF16),
                           ("d_k0", [128, N], BF16), ("d_vT0", [128, 264], BF16),
                           ("d_oall0", [128, N], F32), ("d_rs0", [4, N], F32),
                           ("d_o20", [128, N], BF16), ("d_pe0", [128, N], BF16),
                           ("d_xattn0", [128, N], F32), ("d_negmu2", [1, N], F32),
                           ("d_Arow2", [1, N], F32), ("d_ag0", [128, N], BF16)]:
            dbg[nm] = nc.dram_tensor(nm, sh, dt, kind="ExternalOutput")

    with tile.TileContext(nc) as tc:
        with tc.tile_pool(name="singles", bufs=1) as singles, \
             tc.tile_pool(name="work", bufs=1) as work, \
             tc.tile_pool(name="ptp", bufs=10) as ptp, \
             tc.tile_pool(name="stg", bufs=2) as stg, \
             tc.tile_pool(name="psP", bufs=3, space="PSUM") as psP, \
             tc.tile_pool(name="psO", bufs=1, space="PSUM") as psO:

            # ---- load input first, then constants in consumption order ----
            xt = [work.tile([128, N], F32R, tag=f"x{t}", name=f"x{t}") for t in range(2)]
            for t in range(2):
                for h in range(2):
                    nc.sync.dma_start(
                        xt[t][64 * h: 64 * h + 64, :],
                        x_d.ap()[t * 128 + 64 * h: t * 128 + 64 * h + 64, :])
            early = ["negc", "posc", "epsrow", "ones_rowf", "wqkvT", "wqsumrow",
                     "pvec", "indg", "hw", "hwn", "id128", "tpecol"]
            consts = {}
            by_name = {nm: (nm, sh, dt) for nm, sh, dt in CONST_SPECS}
            order = [by_name[nm] for nm in early] + \
                    [s for s in CONST_SPECS if s[0] not in early]
            for nm, sh, dt in order:
                t = singles.tile(sh, dt, tag=nm, name=nm)
                nc.sync.dma_start(t[:], drams[nm].ap())
                consts[nm] = t

            pv = consts["pvec"]

            # ---- LN1 stats; qkv consumes x directly (no z1 on the critical
            # path): psum = Wq@x + wqsum*(-mu); evac scales by rstd ----
            xsq = [work.tile([128, N], F32R, tag=f"xsq{t}", name=f"xsq{t}")
                   for t in range(2)]
            negmu1, Arow1 = _ln_stats(nc, psP, work, xt, xsq, consts, "l1")

            qk_sb = {}
            vpad = [work.tile([128, 34, 34], F32R, tag=f"pad{t}", name=f"vpad{t}")
                    for t in range(2)]
            for t in range(2):
                nc.gpsimd.memset(vpad[t][:].bitcast(mybir.dt.uint32), 0)
            names6 = ["q0", "k0", "q1", "k1", "v0", "v1"]
            # (tile indices: 2=q1 3=k1 4=v0 5=v1)

            heat_i = [0]

            def emit_heat(n):
                # standalone real-data heater matmuls into the psO slot --
                # output is never read; they only keep the PE HAM window busy
                for _ in range(n):
                    ht = psO.tile([128, 512], F32, tag="po",
                                  name=f"heat{heat_i[0]}")
                    heat_i[0] += 1
                    nc.tensor.matmul(ht[0:64, :], consts["wqkvT"][:, 0, 0:64],
                                     consts["wqkvT"][:, 1, 0:512],
                                     start=True, stop=True)

            def qkv_mm(mt):
                ps = psP.tile([128, N], F32, tag="ps", name=f"qkv{mt}")
                for c in range(2):
                    sl = slice(c * 512, (c + 1) * 512)
                    for kt in range(2):
                        nc.tensor.matmul(
                            ps[:, sl], consts["wqkvT"][:, kt, mt * 128:(mt + 1) * 128],
                            xt[kt][:, sl], start=(kt == 0), stop=False)
                    nc.tensor.matmul(
                        ps[:, sl], consts["wqsumrow"][:, mt * 128:(mt + 1) * 128],
                        negmu1[:, sl], start=False, stop=True)
                return ps

            def qkv_evac(mt, ps):
                nm = names6[mt]
                if nm[0] == "q":
                    t_sb = work.tile([128, N], BF16, tag=nm, name=nm)
                    nc.vector.tensor_tensor(t_sb[:], ps[:], Abc1[:], OP.mult)
                    nc.vector.tensor_scalar(t_sb[:], t_sb[:],
                                            pv[:, int(nm[1]):int(nm[1]) + 1],
                                            None, OP.add)
                    qk_sb[nm] = t_sb
                elif nm[0] == "k":
                    t_sb = work.tile([128, N], BF16, tag=nm, name=nm)
                    nc.vector.tensor_tensor(t_sb[:], ps[:], Abc1[:], OP.mult)
                    qk_sb[nm] = t_sb
                else:
                    vt = int(nm[1])
                    t_sb = work.tile([128, N], BF16, tag=nm, name=nm)
                    nc.vector.tensor_tensor(t_sb[:], ps[:], Abc1[:], OP.mult)
                    qk_sb[nm] = t_sb
                    nc.vector.tensor_scalar(
                        vpad[vt][:, 1:33, 1:33], t_sb[:],
                        pv[:, 2 + vt:3 + vt], None, OP.add)

            # q0/k0 kt-matmuls run while the LN1 row chain computes rstd
            # (their psum groups stay open; the -mu rank row lands later);
            # heaters fill the PE waits so the array stays warm.
            def qkv_kt(mt):
                ps = psP.tile([128, N], F32, tag="ps", name=f"qkv{mt}")
                for c in range(2):
                    sl = slice(c * 512, (c + 1) * 512)
                    for kt in range(2):
                        nc.tensor.matmul(
                            ps[:, sl], consts["wqkvT"][:, kt, mt * 128:(mt + 1) * 128],
                            xt[kt][:, sl], start=(kt == 0), stop=False)
                return ps

            def qkv_rank(mt, ps):
                for c in range(2):
                    sl = slice(c * 512, (c + 1) * 512)
                    nc.tensor.matmul(
                        ps[:, sl], consts["wqsumrow"][:, mt * 128:(mt + 1) * 128],
                        negmu1[:, sl], start=False, stop=True)

            ps_q0 = qkv_kt(0)
            ps_k0 = qkv_kt(1)
            emit_heat(4)
            qkv_rank(0, ps_q0)
            qkv_rank(1, ps_k0)
            Abc1_ps = psP.tile([128, N], F32, tag="ps", name="l1Abc")
            for c in range(2):
                sl = slice(c * 512, (c + 1) * 512)
                nc.tensor.matmul(Abc1_ps[:, sl], consts["ones_rowf"][:, 0:128],
                                 Arow1[:, sl], start=True, stop=True)
            Abc1 = work.tile([128, N], F32R, tag="Abc_sb", name="Abc1")
            nc.scalar.copy(Abc1[:], Abc1_ps[:])
            qkv_evac(0, ps_q0)
            qkv_evac(1, ps_k0)
            for mt in (4, 5):   # v tiles next (vT transposes need them early)
                qkv_evac(mt, qkv_mm(mt))
            # q1/k1 (needed only from pair 2) are spread into pair-0's loop
            qkv_rest = [2, 3]

            def emit_qkv_rest():
                if qkv_rest:
                    mt = qkv_rest.pop(0)
                    qkv_evac(mt, qkv_mm(mt))

            # v^T via PE transpose of v_sb 128x128 blocks (no z1 needed)
            vT_sb = []

            def emit_vT(nt):
                t_sb = work.tile([128, 8, 33], BF16, tag=f"vT{nt}", name=f"vT{nt}")
                for vt in range(2):
                    ps = psP.tile([128, 128], BF16, tag="ps", name=f"vT{nt}{vt}")
                    nc.tensor.transpose(
                        ps[:], qk_sb[f"v{vt}"][:, nt * 128:(nt + 1) * 128],
                        consts["id128"][:])
                    nc.vector.tensor_copy(
                        t_sb[:, 4 * vt: 4 * vt + 4, 0:32], ps[:])
                nc.vector.memset(t_sb[:, :, 32:33], 1.0)
                vT_sb.append(t_sb)

            # ---- attention; pe-dwconv runs on the DVE (PE is the pacer) ----
            pe_sb = [work.tile([128, N], BF16, tag=f"pe{t}", name=f"pe{t}")
                     for t in range(2)]
            pescr = work.tile([128, N], F32, tag="pescr", name="pescr")
            dw_jobs = [(t, tap) for t in range(2) for tap in range(9)]

            def emit_dw(njobs):
                for _ in range(njobs):
                    if not dw_jobs:
                        return
                    t, tap = dw_jobs.pop(0)
                    dy, dx = divmod(tap, 3)
                    shifted = vpad[t][:, dy: dy + 32, dx: dx + 32]
                    tapc = consts["tpecol"][:, t, tap: tap + 1]
                    if tap == 0:
                        nc.vector.tensor_scalar(pescr[:], shifted, tapc,
                                                None, OP.mult)
                    elif tap < 8:
                        nc.vector.scalar_tensor_tensor(
                            pescr[:], shifted, tapc, pescr[:],
                            OP.mult, OP.add)
                    else:
                        nc.vector.scalar_tensor_tensor(
                            pe_sb[t][:], shifted, tapc, pescr[:],
                            OP.mult, OP.add)

            o_all = [work.tile([128, N], F32R, tag=f"oall{g}", name=f"oall{g}")
                     for g in range(2)]
            rs_g = [work.tile([4, N], F32, tag=f"rs{g}", name=f"rs{g}")
                    for g in range(2)]
            o2 = [work.tile([128, N], BF16, tag=f"o2{g}", name=f"o2{g}")
                  for g in range(2)]
            xacc = xt  # proj residual accumulated in place

            pts = {}

            def emit_scores(p, mt, oPS):
                T = p // 2
                jA = (2 * p) % 4
                jg = (jA + 2) % 4   # heater row-group, disjoint from the pair
                q_t, k_t = qk_sb[f"q{T}"], qk_sb[f"k{T}"]
                Ps = [psP.tile([128, N], F32, tag="ps", name=f"sc{p}_{mt}_{hh}")
                      for hh in range(2)]
                for c in range(2):
                    sl = slice(c * 512, (c + 1) * 512)
                    for hh, j in ((0, jA), (1, jA + 1)):
                        nc.tensor.matmul(
                            Ps[hh][:, sl],
                            k_t[32 * j: 32 * j + 16, mt * 128:(mt + 1) * 128],
                            q_t[32 * j: 32 * j + 16, sl],
                            start=True, stop=True, tile_position=(32 * j, 0))
                for hh in range(2):
                    pt = ptp.tile([128, N], BF16, tag="pt", name=f"pt{p}_{mt}_{hh}")
                    nc.scalar.activation(pt[:], Ps[hh][:], AF.Exp)
                    pts[(p, mt, hh)] = pt

            def emit_o(p, mt, oPS):
                hA, hB = 2 * p, 2 * p + 1
                ptA, ptB = pts.pop((p, mt, 0)), pts.pop((p, mt, 1))
                for c in range(2):
                    sl = slice(c * 512, (c + 1) * 512)
                    nc.tensor.matmul(
                        oPS[0:33, sl], vT_sb[mt][:, hA, :], ptA[:, sl],
                        start=(mt == 0), stop=(mt == 7), tile_position=(0, 0))
                    nc.tensor.matmul(
                        oPS[64:97, sl], vT_sb[mt][:, hB, :], ptB[:, sl],
                        start=(mt == 0), stop=(mt == 7), tile_position=(0, 64))

            def emit_recip_o2(g, c):
                # softmax denominators -> recip -> broadcast -> o2 = o * 1/den
                if c == 0:
                    from concourse.dve_ops import (
                        RECIP_APPROX_FAST_CONSTS, RECIPROCAL_APPROX_FAST)
                    cc = RECIP_APPROX_FAST_CONSTS
                    with nc.allow_low_precision(reason="softmax denominators"):
                        nc.vector._custom_dve(
                            RECIPROCAL_APPROX_FAST, out=rs_g[g][:].bitcast(F32R),
                            in0=rs_g[g][:], s0=cc["s0"], s1=cc["s1"],
                            imm2=cc["imm2"])
                sl = slice(c * 512, (c + 1) * 512)
                rB = psP.tile([128, 512], F32, tag="ps", name=f"recipB{g}{c}")
                nc.tensor.matmul(rB[:], consts["indg"][:],
                                 rs_g[g][:, sl].bitcast(F32R),
                                 start=True, stop=True)
                nc.vector.tensor_tensor(o2[g][:, sl], o_all[g][:, sl],
                                        rB[:], OP.mult)

            def emit_proj_g0(mt, c):
                # pe-branch (both kt) + bias + g0's o2 in one accumulation
                sl = slice(c * 512, (c + 1) * 512)
                pp = psP.tile([128, 512], F32, tag="ps", name=f"pj0{mt}{c}")
                mc = slice(mt * 128, (mt + 1) * 128)
                nc.tensor.matmul(pp[:], consts["wprojT"][:, 0, mc],
                                 pe_sb[0][:, sl], start=True, stop=False)
                nc.tensor.matmul(pp[:], consts["wprojT"][:, 1, mc],
                                 pe_sb[1][:, sl], start=False, stop=False)
                nc.tensor.matmul(pp[:], consts["bprojrow"][:, mc],
                                 consts["ones_rowf"][:, 0:512],
                                 start=False, stop=False)
                nc.tensor.matmul(pp[:], consts["wprojT"][:, 0, mc],
                                 o2[0][:, sl], start=False, stop=True)
                nc.vector.tensor_tensor(xacc[mt][:, sl], xacc[mt][:, sl],
                                        pp[:], OP.add)

            def emit_proj_g1(mt, c):
                sl = slice(c * 512, (c + 1) * 512)
                pp = psP.tile([128, 512], F32, tag="ps", name=f"pj1{mt}{c}")
                nc.tensor.matmul(pp[:], consts["wprojT"][:, 1,
                                                         mt * 128:(mt + 1) * 128],
                                 o2[1][:, sl], start=True, stop=True)
                nc.vector.tensor_tensor(xacc[mt][:, sl], xacc[mt][:, sl],
                                        pp[:], OP.add)

            def emit_group_mid0_part(i):
                if i == 0:
                    emit_recip_o2(0, 0)
                elif i == 1:
                    emit_recip_o2(0, 1)
                elif i < 6:
                    emit_proj_g0((i - 2) // 2, (i - 2) % 2)

            def emit_group_mid(g):
                assert g == 1
                for c in range(2):
                    emit_recip_o2(1, c)
                for mt in range(2):
                    for c in range(2):
                        sl = slice(c * 512, (c + 1) * 512)
                        pp = psP.tile([128, 512], F32, tag="ps",
                                      name=f"pj1{mt}{c}")
                        nc.tensor.matmul(
                            pp[:], consts["wprojT"][:, 1,
                                                    mt * 128:(mt + 1) * 128],
                            o2[1][:, sl], start=True, stop=True)
                        nc.vector.tensor_tensor(xacc[mt][:, sl],
                                                xacc[mt][:, sl],
                                                pp[:], OP.add)

            for p in range(4):
                g = p // 2
                oPS = psO.tile([128, N], F32, tag="po", name=f"oacc{p}")
                for mt in range(8):
                    emit_scores(p, mt, oPS)
                    if p == 0:
                        emit_vT(mt)
                    if mt > 0:
                        # cancelling heater pair emitted while the PE would
                        # otherwise wait on the exp gating o(mt-1); pair 0's
                        # PE is already oversubscribed, so skip it there
                        for wname in ("hw", "hwn"):
                            nc.tensor.matmul(
                                oPS[0:33, 0:512], consts[wname][:, 0:33],
                                qk_sb["q0"][:, 0:512], start=False, stop=False)
                        emit_o(p, mt - 1, oPS)
                    if p == 1:
                        emit_dw(2)
                        emit_qkv_rest()
                    elif p == 2:
                        emit_dw(1)
                    # g0's mid work spread over p3's slots (deps long ready)
                    if p == 3 and 1 <= mt <= 4:
                        emit_group_mid0_part(mt - 1)
                if p == 1:
                    emit_dw(2)
                if p == 3:
                    # last two proj parts ride the tail-exp wait
                    emit_group_mid0_part(4)
                    emit_group_mid0_part(5)
                emit_o(p, 7, oPS)
                stage = stg.tile([97, N], F32R, tag="stage", name=f"stage{p}")
                nc.vector.tensor_copy(stage[:], oPS[0:97, :])
                pp = p % 2
                nc.sync.dma_start(o_all[g][64 * pp: 64 * pp + 32, :], stage[0:32, :])
                nc.sync.dma_start(o_all[g][64 * pp + 32: 64 * pp + 64, :],
                                  stage[64:96, :])
                nc.sync.dma_start(rs_g[g][2 * pp: 2 * pp + 1, :],
                                  stage[32:33, :].bitcast(F32))
                nc.sync.dma_start(rs_g[g][2 * pp + 1: 2 * pp + 2, :],
                                  stage[96:97, :].bitcast(F32))

            emit_group_mid(1)
            x_attn = xacc

            if debug_outs:
                nc.sync.dma_start(dbg["d_q0"].ap(), qk_sb["q0"][:])
                nc.sync.dma_start(dbg["d_k0"].ap(), qk_sb["k0"][:])
                nc.sync.dma_start(dbg["d_vT0"].ap(), vT_sb[0][:])
                nc.sync.dma_start(dbg["d_oall0"].ap(), o_all[0][:].bitcast(F32))
                nc.sync.dma_start(dbg["d_rs0"].ap(), rs_g[0][:])
                nc.sync.dma_start(dbg["d_o20"].ap(), o2[0][:])
                nc.sync.dma_start(dbg["d_pe0"].ap(), pe_sb[0][:])
                nc.sync.dma_start(dbg["d_xattn0"].ap(), x_attn[0][:].bitcast(F32))

            # ---- LN2 stats (no z2 materialization) ----
            negmu2, Arow2 = _ln_stats(nc, psP, work, x_attn, xsq, consts, "l2")
            B2row = work.tile([1, N], F32R, tag="ln_B", name="B2row")
            nc.vector.tensor_tensor(B2row[:], negmu2[:], Arow2[:], OP.mult)

            if debug_outs:
                nc.sync.dma_start(dbg["d_negmu2"].ap(), negmu2[:].bitcast(F32))
                nc.sync.dma_start(dbg["d_Arow2"].ap(), Arow2[:].bitcast(F32))

            # ---- fc1: psum = W1@x + w1sum*(-mu); evac scales by rstd ----
            apad = [work.tile([128, 34, 34], F32R, tag=f"pad{t}", name=f"apad{t}")
                    for t in range(2)]
            for t in range(2):
                nc.gpsimd.memset(apad[t][:].bitcast(mybir.dt.uint32), 0)
            Gtmp = [work.tile([128, N], BF16, tag=f"Gt{t}", name=f"Gt{t}")
                    for t in range(2)]
            nparts = [128, 42, 128, 42]

            def fc1_mm(mt, np_, with_rank):
                ps = psP.tile([128, N], F32, tag="ps", name=f"fc1_{mt}")
                for c in range(2):
                    sl = slice(c * 512, (c + 1) * 512)
                    for kt in range(2):
                        nc.tensor.matmul(
                            ps[0:np_, sl],
                            consts["wfc1T"][:, kt, mt * 128: mt * 128 + np_],
                            x_attn[kt][:, sl], start=(kt == 0),
                            stop=(kt == 1 and not with_rank))
                    if with_rank:
                        nc.tensor.matmul(
                            ps[0:np_, sl],
                            consts["w1sumrow"][:, mt * 128: mt * 128 + np_],
                            negmu2[:, sl], start=False, stop=True)
                return ps

            def fc1_rank(mt, np_, ps):
                for c in range(2):
                    sl = slice(c * 512, (c + 1) * 512)
                    nc.tensor.matmul(
                        ps[0:np_, sl],
                        consts["w1sumrow"][:, mt * 128: mt * 128 + np_],
                        negmu2[:, sl], start=False, stop=True)

            def fc1_evac(mt, np_, ps):
                if mt < 2:
                    nc.vector.tensor_tensor(apad[mt][0:np_, 1:33, 1:33],
                                            ps[0:np_], Abc2[0:np_], OP.mult)
                    nc.vector.tensor_scalar(apad[mt][0:np_, 1:33, 1:33],
                                            apad[mt][0:np_, 1:33, 1:33],
                                            pv[0:np_, 8 + mt:9 + mt], None, OP.add)
                else:
                    nc.vector.tensor_tensor(Gtmp[mt - 2][0:np_], ps[0:np_],
                                            Abc2[0:np_], OP.mult)

            # A1/A2 kt-matmuls overlap the LN2 row chain (open psum groups),
            # then broadcast rstd, finish with the rank rows, evacuate.
            ps_a1 = fc1_mm(0, 128, with_rank=False)
            ps_a2 = fc1_mm(1, 42, with_rank=False)
            emit_heat(8)
            Abc2_ps = psP.tile([128, N], F32, tag="ps", name="l2Abc")
            for c in range(2):
                sl = slice(c * 512, (c + 1) * 512)
                nc.tensor.matmul(Abc2_ps[:, sl], consts["ones_rowf"][:, 0:128],
                                 Arow2[:, sl], start=True, stop=True)
            Abc2 = work.tile([128, N], F32R, tag="Abc_sb", name="Abc2")
            nc.scalar.copy(Abc2[:], Abc2_ps[:])
            fc1_rank(1, 42, ps_a2)
            fc1_rank(0, 128, ps_a1)
            fc1_evac(1, 42, ps_a2)
            fc1_evac(0, 128, ps_a1)
            for mt in (2, 3):
                np_ = nparts[mt]
                ps = fc1_mm(mt, np_, with_rank=True)
                fc1_evac(mt, np_, ps)

            # W = Abc2 * x_attn feeds y1 = (W*g2 + bfin) + x_attn (DVE, runs
            # while the GLU dwconv occupies the PE; needed only at fc2 tail)
            W = [work.tile([128, N], F32R, tag=f"xsq{t}", name=f"W{t}") for t in range(2)]
            y1 = [work.tile([128, N], F32, tag=f"oall{t}", name=f"y1{t}")
                  for t in range(2)]

            # ---- GLU dwconv on the PE (runs warm behind heaters) ----
            dwps = []
            for t in range(2):
                np_ = nparts[t]
                ps = psP.tile([128, N], F32, tag="ps", name=f"dwglu{t}")
                for c in range(2):
                    for tap in range(9):
                        dy, dx = divmod(tap, 3)
                        rhs = apad[t][0:np_, dy + 16 * c: dy + 16 * c + 16, dx: dx + 32]
                        nc.tensor.matmul(
                            ps[0:np_, c * 512:(c + 1) * 512],
                            consts["ddw"][0:np_, t, tap, 0:np_], rhs,
                            start=(tap == 0), stop=(tap == 8))
                dwps.append(ps)
            ag = []
            for t in range(2):
                np_ = nparts[t]
                a_act = work.tile([128, N], BF16, tag=f"aact{t}", name=f"aact{t}")
                nc.scalar.activation(a_act[0:np_], dwps[t][0:np_], AF.Gelu,
                                     bias=pv[0:np_, 4 + t:5 + t])
                # W/y1 DVE work overlaps the PE dwconv / ACT gelu
                nc.vector.tensor_tensor(W[t][:], x_attn[t][:], Abc2[:], OP.mult)
                nc.vector.affine_then_add(y1[t][:], W[t][:].bitcast(F32),
                                          x_attn[t][:].bitcast(F32),
                                          pv[:, 12 + t:13 + t], pv[:, 6 + t:7 + t])
                agt = work.tile([128, N], BF16, tag=f"ag{t}", name=f"ag{t}")
                nc.vector.scalar_tensor_tensor(
                    agt[0:np_], Gtmp[t][0:np_], pv[0:np_, 10 + t:11 + t],
                    a_act[0:np_], OP.add, OP.mult)
                ag.append(agt)

            if debug_outs:
                nc.sync.dma_start(dbg["d_ag0"].ap(), ag[0][:])

            # ---- fc2 (+ rank-1 g2 x B2) + final residual ----
            # kt0 contribution issues as soon as ag[0] exists; kt1 + the
            # rank row close the accumulation once ag[1] lands.
            fc2_ps = [psP.tile([128, N], F32, tag="ps", name=f"fc2_{mt}")
                      for mt in range(2)]
            for mt in range(2):
                for c in range(2):
                    sl = slice(c * 512, (c + 1) * 512)
                    nc.tensor.matmul(
                        fc2_ps[mt][:, sl],
                        consts["wfc2T"][0:128, 0, mt * 128:(mt + 1) * 128],
                        ag[0][0:128, sl], start=True, stop=False)
            for mt in range(2):
                for c in range(2):
                    sl = slice(c * 512, (c + 1) * 512)
                    nc.tensor.matmul(
                        fc2_ps[mt][:, sl],
                        consts["wfc2T"][0:42, 1, mt * 128:(mt + 1) * 128],
                        ag[1][0:42, sl], start=False, stop=False)
                    nc.tensor.matmul(
                        fc2_ps[mt][:, sl],
                        consts["g2row"][:, mt * 128:(mt + 1) * 128],
                        B2row[:, sl], start=False, stop=True)
                yt = work.tile([128, N], F32, tag=f"xsq{mt}", name=f"y{mt}")
                nc.vector.tensor_tensor(yt[:], y1[mt][:], fc2_ps[mt][:], OP.add)
                nc.sync.dma_start(y_d.ap()[mt * 128:(mt + 1) * 128, :], yt[:])

    nc.compile()
    return nc


_NC = None
_NC_DBG = None


def kernel(**inputs):
    global _NC
    consts = fold_consts(inputs)
    if _NC is None:
        _NC = build()
    x = np.asarray(inputs["x"], np.float32)
    B = x.shape[0]
    in_maps = []
    for b in range(B):
        m = dict(consts)
        m["x"] = np.ascontiguousarray(x[b].reshape(C, N))
        in_maps.append(m)
    res = run_bass_kernel_spmd(_NC, in_maps, core_ids=list(range(B)))
    out = np.stack([res.results[b]["y"].reshape(C, HH, WW) for b in range(B)])
    return out
